# revision 5
# baseline (speedup 1.0000x reference)
"""HDCAM dense-transformer Trainium2 kernel.

Sharding: data-parallel over B across the 8 NeuronCores (B == 8, one batch
element per core).  All weights are replicated; no collectives.

Per-core pipeline (T=1024, V=256, D=4096, K=8192, H=512):
  P0  one-hot(idx) on device (iota + is_equal), chars = onehot^T.T @ codebook
      (PE, bf16 exact since codebook is +-1), trigram hdc via shifted
      SBUF->SBUF DMAs + DVE elementwise.
  P1  ctx^T[d,t] = (hdc[s,d])^T-contract via PE: lhsT=hdc[s,d] rhs=dm^T[s,t].
  P2  flash-style softmax over K with no max subtraction (logits provably
      in [-0.1, 0.1]): scores^T[k,t] = keys^T[d,k].T @ ctx^T[d,t], exp on ACT
      (scale 1/sqrt(D)), retrieved[t,h] += p^T.T @ values, denom via ones
      column, all accumulated in PSUM across the 64 k-tiles.
  P2.5 hdc_float = ctx @ proj_w (+proj_b via rank-1 ones matmul).
  P2.7 residual = onehot^T.T @ res_embed + pos_embed.
  P3  LayerNorm over 3H (ln_g/ln_b folded into w1/b1 on host), PE-transpose
      of z, 3x erf-gelu MLP in f32r (TF32-rate), head matmul + bias.

dtypes: big matmuls bf16 x bf16 (HW forbids mixing 16/32-bit operands);
MLP tail f32r x f32r (1 cycle/row at N>=256, ~1e-4 precision).
"""
import numpy as np
import ml_dtypes

B, T, V, D, K, H = 8, 1024, 256, 4096, 8192, 512
DECAY = 0.95
LN_EPS = 1e-5
N_CORES = 8

TT = T // 128    # 8  t-tiles
DT = D // 128    # 32 d-tiles
KT = K // 128    # 64 k-tiles
VT = V // 128    # 2  v-tiles
CT = (3 * H) // 128  # 12 contraction tiles for w1
HT = H // 128    # 4  h-tiles
TH = 2           # t-halves (PSUM capacity)
THS = T // TH    # 512

BF16 = ml_dtypes.bfloat16

_CACHE = {}


def _build_program():
    from contextlib import ExitStack
    import concourse.bacc as bacc
    import concourse.tile as tile
    import concourse.mybir as mybir
    from concourse.masks import make_identity

    f32 = mybir.dt.float32
    f32r = mybir.dt.float32r
    bf16 = mybir.dt.bfloat16
    i32 = mybir.dt.int32
    Alu = mybir.AluOpType
    Act = mybir.ActivationFunctionType

    nc = bacc.Bacc("TRN2", target_bir_lowering=False, debug=False,
                   num_devices=N_CORES)

    # ---- DRAM I/O ----
    idx_d = nc.dram_tensor("idx", (1, T), i32, kind="ExternalInput")
    cb_d = nc.dram_tensor("cb", (V, D), bf16, kind="ExternalInput")
    keys_d = nc.dram_tensor("keys", (K, D), bf16, kind="ExternalInput")  # pre-tiled
    vals_d = nc.dram_tensor("vals", (K, H), bf16, kind="ExternalInput")
    dmT_d = nc.dram_tensor("dmT", (T, T), bf16, kind="ExternalInput")
    projw_d = nc.dram_tensor("projw", (D, H), bf16, kind="ExternalInput")
    projb_d = nc.dram_tensor("projb", (1, H), bf16, kind="ExternalInput")
    res_d = nc.dram_tensor("rese", (V, H), bf16, kind="ExternalInput")
    pos_d = nc.dram_tensor("pos", (T, H), f32, kind="ExternalInput")
    w1_d = nc.dram_tensor("w1", (3 * H, H), f32r, kind="ExternalInput")
    b1_d = nc.dram_tensor("b1", (128, HT), f32, kind="ExternalInput")
    w2_d = nc.dram_tensor("w2", (H, H), f32r, kind="ExternalInput")
    b2_d = nc.dram_tensor("b2", (128, HT), f32, kind="ExternalInput")
    w3_d = nc.dram_tensor("w3", (H, H), f32r, kind="ExternalInput")
    b3_d = nc.dram_tensor("b3", (128, HT), f32, kind="ExternalInput")
    hw_d = nc.dram_tensor("hw", (H, V), f32r, kind="ExternalInput")
    hb_d = nc.dram_tensor("hb", (1, V), f32r, kind="ExternalInput")
    out_d = nc.dram_tensor("out", (T, V), f32, kind="ExternalOutput")
    x_d = nc.dram_tensor("xbuf", (T, 3 * H), f32, kind="Internal")

    with tile.TileContext(nc) as tc:
        with ExitStack() as top:
            # ======== persistent pools ========
            oh_pool = top.enter_context(tc.tile_pool(name="oh", bufs=1))
            onehot = [oh_pool.tile([128, T], bf16, name=f"onehot{v}")
                      for v in range(VT)]
            hdc_pool = tc.alloc_tile_pool(name="hdc", bufs=1)
            hdc = [hdc_pool.tile([128, D], bf16, name=f"hdc{i}")
                   for i in range(TT)]

            # ======== P0a: one-hot ========
            with tc.tile_pool(name="p0a", bufs=1) as p0a:
                idx_b = p0a.tile([128, T], i32)
                nc.sync.dma_start(out=idx_b, in_=idx_d.ap().partition_broadcast(128))
                idx_f = p0a.tile([128, T], f32)
                nc.vector.tensor_copy(out=idx_f, in_=idx_b)
                iota_i = p0a.tile([128, 1], i32)
                nc.gpsimd.iota(iota_i, pattern=[[0, 1]], base=0, channel_multiplier=1)
                iota_f = [p0a.tile([128, 1], f32, name=f"iota_f{v}") for v in range(VT)]
                nc.vector.tensor_copy(out=iota_f[0], in_=iota_i)
                nc.vector.tensor_scalar(out=iota_f[1], in0=iota_f[0], scalar1=128.0,
                                        scalar2=None, op0=Alu.add)
                for v in range(VT):
                    nc.vector.tensor_scalar(out=onehot[v], in0=idx_f,
                                            scalar1=iota_f[v], scalar2=None,
                                            op0=Alu.is_equal)

            # ======== P0b: chars + trigram hdc ========
            with ExitStack() as p0ctx:
                cb_pool = p0ctx.enter_context(tc.tile_pool(name="cbp", bufs=1))
                cb_sb = [cb_pool.tile([128, D], bf16, name=f"cb{v}") for v in range(VT)]
                for v in range(VT):
                    nc.sync.dma_start(out=cb_sb[v], in_=cb_d.ap()[v * 128:(v + 1) * 128, :])
                chars_pool = p0ctx.enter_context(tc.tile_pool(name="chars", bufs=3))
                cm_pool = p0ctx.enter_context(tc.tile_pool(name="cm", bufs=2))
                cps = p0ctx.enter_context(tc.tile_pool(name="cps", bufs=2, space="PSUM"))

                prev_chars = None
                for i in range(TT):
                    chars_i = chars_pool.tile([128, D], bf16, tag="chars")
                    for dc in range(D // 512):
                        cp = cps.tile([128, 512], f32, tag="cp")
                        for v in range(VT):
                            nc.tensor.matmul(cp, lhsT=onehot[v][:, i * 128:(i + 1) * 128],
                                             rhs=cb_sb[v][:, dc * 512:(dc + 1) * 512],
                                             start=(v == 0), stop=(v == VT - 1))
                        nc.scalar.copy(out=chars_i[:, dc * 512:(dc + 1) * 512], in_=cp)
                    # cm1: rows shifted down 1 (t-1), free rolled by 1
                    cm1 = cm_pool.tile([128, D], bf16, tag="cm1")
                    nc.sync.dma_start(out=cm1[1:128, 1:D], in_=chars_i[0:127, 0:D - 1])
                    nc.sync.dma_start(out=cm1[1:128, 0:1], in_=chars_i[0:127, D - 1:D])
                    if i > 0:
                        nc.sync.dma_start(out=cm1[0:1, 1:D], in_=prev_chars[127:128, 0:D - 1])
                        nc.sync.dma_start(out=cm1[0:1, 0:1], in_=prev_chars[127:128, D - 1:D])
                    else:
                        nc.vector.memset(cm1[0:1, :], 0.0)
                    # cm2: rows shifted down 2 (t-2), free rolled by 2
                    cm2 = cm_pool.tile([128, D], bf16, tag="cm2")
                    nc.sync.dma_start(out=cm2[2:128, 2:D], in_=chars_i[0:126, 0:D - 2])
                    nc.sync.dma_start(out=cm2[2:128, 0:2], in_=chars_i[0:126, D - 2:D])
                    if i > 0:
                        nc.sync.dma_start(out=cm2[0:2, 2:D], in_=prev_chars[126:128, 0:D - 2])
                        nc.sync.dma_start(out=cm2[0:2, 0:2], in_=prev_chars[126:128, D - 2:D])
                    else:
                        nc.vector.memset(cm2[0:2, :], 0.0)
                    # hdc = 0.7 * chars*cm1*cm2 + 0.3 * chars
                    nc.vector.tensor_mul(out=cm1, in0=cm1, in1=cm2)
                    nc.vector.scalar_tensor_tensor(out=cm1, in0=cm1, scalar=0.7,
                                                   in1=chars_i, op0=Alu.mult, op1=Alu.mult)
                    nc.vector.scalar_tensor_tensor(out=hdc[i], in0=chars_i, scalar=0.3,
                                                   in1=cm1, op0=Alu.mult, op1=Alu.add)
                    prev_chars = chars_i

            # ======== P1: ctx^T = hdc^T @ dm^T ========
            ctx_pool = tc.alloc_tile_pool(name="ctx", bufs=1)
            ctx = [ctx_pool.tile([128, T], bf16, name=f"ctx{i}")
                   for i in range(DT)]
            with ExitStack() as p1ctx:
                dm_pool = p1ctx.enter_context(tc.tile_pool(name="dmp", bufs=1))
                dm_sb = [dm_pool.tile([128, T], bf16, name=f"dm{s}") for s in range(TT)]
                for s in range(TT):
                    nc.sync.dma_start(out=dm_sb[s], in_=dmT_d.ap()[s * 128:(s + 1) * 128, :])
                xps = p1ctx.enter_context(tc.tile_pool(name="xps", bufs=4, space="PSUM"))
                for dt in range(DT):
                    for th in range(TH):
                        smax = (th + 1) * (TT // TH)  # causal: s-tiles 0..smax-1
                        pc = xps.tile([128, THS], f32, tag="pc")
                        for s in range(smax):
                            nc.tensor.matmul(pc, lhsT=hdc[s][:, dt * 128:(dt + 1) * 128],
                                             rhs=dm_sb[s][:, th * THS:(th + 1) * THS],
                                             start=(s == 0), stop=(s == smax - 1))
                        nc.scalar.copy(out=ctx[dt][:, th * THS:(th + 1) * THS], in_=pc)
            # ======== P2: associative memory (flash softmax over K) ========
            with ExitStack() as p2ctx:
                kv_pool = p2ctx.enter_context(tc.tile_pool(name="kvp", bufs=3))
                p_pool = p2ctx.enter_context(tc.tile_pool(name="pp", bufs=3))
                one_pool = p2ctx.enter_context(tc.tile_pool(name="onep", bufs=1))
                ones_col = one_pool.tile([128, 1], bf16)
                nc.vector.memset(ones_col, 1.0)
                scps = p2ctx.enter_context(tc.tile_pool(name="scps", bufs=2, space="PSUM"))
                retps = p2ctx.enter_context(tc.tile_pool(name="retps", bufs=1, space="PSUM"))
                nrm_pool = p2ctx.enter_context(tc.tile_pool(name="nrm", bufs=2))
                for th in range(TH):
                    ret = [retps.tile([128, H], f32, tag="ret", name=f"ret{th}_{ts}", bufs=4)
                           for ts in range(4)]
                    den = retps.tile([128, 4], f32, tag="den", name=f"den{th}", bufs=1)
                    for kt in range(KT):
                        keys_t = kv_pool.tile([128, D], bf16, tag="keys")
                        nc.sync.dma_start(out=keys_t, in_=keys_d.ap()[kt * 128:(kt + 1) * 128, :])
                        vals_t = kv_pool.tile([128, H], bf16, tag="vals")
                        nc.sync.dma_start(out=vals_t, in_=vals_d.ap()[kt * 128:(kt + 1) * 128, :])
                        sc = scps.tile([128, THS], f32, tag="sc")
                        for dt in range(DT):
                            nc.tensor.matmul(sc, lhsT=keys_t[:, dt * 128:(dt + 1) * 128],
                                             rhs=ctx[dt][:, th * THS:(th + 1) * THS],
                                             start=(dt == 0), stop=(dt == DT - 1))
                        p_t = p_pool.tile([128, THS], bf16, tag="p")
                        nc.scalar.activation(out=p_t, in_=sc, func=Act.Exp,
                                             scale=float(D) ** -0.5)
                        for ts in range(4):
                            nc.tensor.matmul(ret[ts], lhsT=p_t[:, ts * 128:(ts + 1) * 128],
                                             rhs=vals_t, start=(kt == 0), stop=(kt == KT - 1))
                            nc.tensor.matmul(den[:, ts:ts + 1],
                                             lhsT=p_t[:, ts * 128:(ts + 1) * 128],
                                             rhs=ones_col, start=(kt == 0), stop=(kt == KT - 1))
                    recip = nrm_pool.tile([128, 4], f32, tag="recip")
                    nc.vector.reciprocal(out=recip, in_=den)
                    for ts in range(4):
                        gt = th * 4 + ts
                        xw = nrm_pool.tile([128, H], f32, tag="xw")
                        nc.vector.tensor_scalar(out=xw, in0=ret[ts],
                                                scalar1=recip[:, ts:ts + 1], scalar2=None,
                                                op0=Alu.mult)
                        nc.sync.dma_start(out=x_d.ap()[gt * 128:(gt + 1) * 128, H:2 * H],
                                          in_=xw)

            # ======== P2.5: hdc_float = ctx @ proj_w + proj_b ========
            with ExitStack() as p25:
                pw_pool = p25.enter_context(tc.tile_pool(name="pwp", bufs=1))
                pw_sb = [pw_pool.tile([128, H], bf16, name=f"pw{dt}") for dt in range(DT)]
                for dt in range(DT):
                    nc.sync.dma_start(out=pw_sb[dt], in_=projw_d.ap()[dt * 128:(dt + 1) * 128, :])
                pb_sb = pw_pool.tile([1, H], bf16)
                nc.sync.dma_start(out=pb_sb, in_=projb_d.ap())
                ones_row = pw_pool.tile([1, 128], bf16)
                nc.vector.memset(ones_row, 1.0)
                pjps = p25.enter_context(tc.tile_pool(name="pjps", bufs=2, space="PSUM"))
                for tt in range(TT):
                    pj = pjps.tile([128, H], f32, tag="pj")
                    for dt in range(DT):
                        nc.tensor.matmul(pj, lhsT=ctx[dt][:, tt * 128:(tt + 1) * 128],
                                         rhs=pw_sb[dt], start=(dt == 0), stop=False)
                    nc.tensor.matmul(pj, lhsT=ones_row, rhs=pb_sb, start=False, stop=True)
                    xw = pw_pool.tile([128, H], f32, tag="pxw", bufs=3)
                    nc.vector.tensor_copy(out=xw, in_=pj)
                    nc.sync.dma_start(out=x_d.ap()[tt * 128:(tt + 1) * 128, 0:H], in_=xw)
            ctx_pool.release()
            hdc_pool.release()

            # ======== P2.7: residual = res_embed[idx] + pos_embed ========
            with ExitStack() as p27:
                re_pool = p27.enter_context(tc.tile_pool(name="rep", bufs=1))
                re_sb = [re_pool.tile([128, H], bf16, name=f"re{v}") for v in range(VT)]
                for v in range(VT):
                    nc.sync.dma_start(out=re_sb[v], in_=res_d.ap()[v * 128:(v + 1) * 128, :])
                pos_pool = p27.enter_context(tc.tile_pool(name="posp", bufs=2))
                rsps = p27.enter_context(tc.tile_pool(name="rsps", bufs=2, space="PSUM"))
                for tt in range(TT):
                    rs = rsps.tile([128, H], f32, tag="rs")
                    for v in range(VT):
                        nc.tensor.matmul(rs, lhsT=onehot[v][:, tt * 128:(tt + 1) * 128],
                                         rhs=re_sb[v], start=(v == 0), stop=(v == VT - 1))
                    pos_t = pos_pool.tile([128, H], f32, tag="pos")
                    nc.sync.dma_start(out=pos_t, in_=pos_d.ap()[tt * 128:(tt + 1) * 128, :])
                    xw = pos_pool.tile([128, H], f32, tag="rxw")
                    nc.vector.tensor_add(out=xw, in0=rs, in1=pos_t)
                    nc.sync.dma_start(out=x_d.ap()[tt * 128:(tt + 1) * 128, 2 * H:3 * H],
                                      in_=xw)

            # ======== P3: LN + MLP + head ========
            with ExitStack() as p3:
                w_pool = p3.enter_context(tc.tile_pool(name="wp", bufs=1))
                w1_sb = [w_pool.tile([128, H], f32r, name=f"w1_{c}") for c in range(CT)]
                for c in range(CT):
                    nc.sync.dma_start(out=w1_sb[c], in_=w1_d.ap()[c * 128:(c + 1) * 128, :])
                w2_sb = [w_pool.tile([128, H], f32r, name=f"w2_{c}") for c in range(HT)]
                w3_sb = [w_pool.tile([128, H], f32r, name=f"w3_{c}") for c in range(HT)]
                hw_sb = [w_pool.tile([128, V], f32r, name=f"hw_{c}") for c in range(HT)]
                for c in range(HT):
                    nc.sync.dma_start(out=w2_sb[c], in_=w2_d.ap()[c * 128:(c + 1) * 128, :])
                    nc.sync.dma_start(out=w3_sb[c], in_=w3_d.ap()[c * 128:(c + 1) * 128, :])
                    nc.sync.dma_start(out=hw_sb[c], in_=hw_d.ap()[c * 128:(c + 1) * 128, :])
                b1_sb = w_pool.tile([128, HT], f32)
                b2_sb = w_pool.tile([128, HT], f32)
                b3_sb = w_pool.tile([128, HT], f32)
                nc.sync.dma_start(out=b1_sb, in_=b1_d.ap())
                nc.sync.dma_start(out=b2_sb, in_=b2_d.ap())
                nc.sync.dma_start(out=b3_sb, in_=b3_d.ap())
                hb_sb = w_pool.tile([1, V], f32r)
                nc.sync.dma_start(out=hb_sb, in_=hb_d.ap())
                ones_rf = w_pool.tile([1, 128], f32)
                nc.vector.memset(ones_rf, 1.0)
                ones_r = w_pool.tile([1, 128], f32r)
                nc.vector.tensor_copy(out=ones_r, in_=ones_rf)
                eps_sb = w_pool.tile([128, 1], f32)
                nc.vector.memset(eps_sb, LN_EPS)
                ident = w_pool.tile([128, 128], f32)
                make_identity(nc, ident)

                zT_pool = p3.enter_context(tc.tile_pool(name="zTp", bufs=1))
                ln_pool = p3.enter_context(tc.tile_pool(name="lnp", bufs=3))
                h_pool = p3.enter_context(tc.tile_pool(name="hp", bufs=1))
                o_pool = p3.enter_context(tc.tile_pool(name="op", bufs=2))
                tpps = p3.enter_context(tc.tile_pool(name="tpps", bufs=2, space="PSUM"))
                mlpps = p3.enter_context(tc.tile_pool(name="mlpps", bufs=4, space="PSUM"))
                outps = p3.enter_context(tc.tile_pool(name="outps", bufs=2, space="PSUM"))

                for ch in range(TH):  # two chunks of 512 tokens
                    zT = [zT_pool.tile([128, THS], f32r, tag=f"zT{c}", name=f"zT{ch}_{c}")
                          for c in range(CT)]
                    for tl in range(4):
                        tt = ch * 4 + tl
                        x_t = ln_pool.tile([128, 3 * H], f32, tag="xln")
                        nc.sync.dma_start(out=x_t,
                                          in_=x_d.ap()[tt * 128:(tt + 1) * 128, :])
                        stats = ln_pool.tile([128, 3, 6], f32, tag="stats")
                        for j in range(3):
                            nc.vector.bn_stats(out=stats[:, j, :],
                                               in_=x_t[:, j * 512:(j + 1) * 512])
                        mv = ln_pool.tile([128, 2], f32, tag="mv")
                        nc.vector.bn_aggr(out=mv, in_=stats)
                        sd = ln_pool.tile([128, 1], f32, tag="sd")
                        nc.scalar.activation(out=sd, in_=mv[:, 1:2], func=Act.Sqrt,
                                             bias=eps_sb)
                        rstd = ln_pool.tile([128, 1], f32, tag="rstd")
                        nc.vector.reciprocal(out=rstd, in_=sd)
                        z_t = ln_pool.tile([128, 3 * H], f32, tag="z")
                        nc.vector.tensor_scalar(out=z_t, in0=x_t, scalar1=mv[:, 0:1],
                                                scalar2=rstd, op0=Alu.subtract,
                                                op1=Alu.mult)
                        for c in range(CT):
                            tp = tpps.tile([128, 128], f32, tag="tp")
                            nc.tensor.transpose(tp, z_t[:, c * 128:(c + 1) * 128], ident)
                            nc.vector.tensor_copy(out=zT[c][:, tl * 128:(tl + 1) * 128],
                                                  in_=tp)
                    # --- MLP on this 512-token chunk, all in [h, t] layout ---
                    h1 = [h_pool.tile([128, THS], f32r, tag=f"h1_{hi}",
                                      name=f"h1_{ch}_{hi}") for hi in range(HT)]
                    for hi in range(HT):
                        mp = mlpps.tile([128, THS], f32, tag="mp", name=f"mp1_{ch}_{hi}")
                        for c in range(CT):
                            nc.tensor.matmul(mp, lhsT=w1_sb[c][:, hi * 128:(hi + 1) * 128],
                                             rhs=zT[c], start=(c == 0), stop=(c == CT - 1))
                        nc.scalar.activation(out=h1[hi], in_=mp, func=Act.Gelu,
                                             bias=b1_sb[:, hi:hi + 1])
                    h2 = [h_pool.tile([128, THS], f32r, tag=f"h2_{hi}",
                                      name=f"h2_{ch}_{hi}") for hi in range(HT)]
                    for hi in range(HT):
                        mp = mlpps.tile([128, THS], f32, tag="mp", name=f"mp2_{ch}_{hi}")
                        for c in range(HT):
                            nc.tensor.matmul(mp, lhsT=w2_sb[c][:, hi * 128:(hi + 1) * 128],
                                             rhs=h1[c], start=(c == 0), stop=(c == HT - 1))
                        nc.scalar.activation(out=h2[hi], in_=mp, func=Act.Gelu,
                                             bias=b2_sb[:, hi:hi + 1])
                    h3 = [h_pool.tile([128, THS], f32r, tag=f"h3_{hi}",
                                      name=f"h3_{ch}_{hi}") for hi in range(HT)]
                    for hi in range(HT):
                        mp = mlpps.tile([128, THS], f32, tag="mp", name=f"mp3_{ch}_{hi}")
                        for c in range(HT):
                            nc.tensor.matmul(mp, lhsT=w3_sb[c][:, hi * 128:(hi + 1) * 128],
                                             rhs=h2[c], start=(c == 0), stop=(c == HT - 1))
                        nc.scalar.activation(out=h3[hi], in_=mp, func=Act.Gelu,
                                             bias=b3_sb[:, hi:hi + 1])
                    for tl in range(4):
                        op = outps.tile([128, V], f32, tag="opx")
                        for hi in range(HT):
                            nc.tensor.matmul(op, lhsT=h3[hi][:, tl * 128:(tl + 1) * 128],
                                             rhs=hw_sb[hi], start=(hi == 0), stop=False)
                        nc.tensor.matmul(op, lhsT=ones_r, rhs=hb_sb, start=False, stop=True)
                        o_t = o_pool.tile([128, V], f32, tag="ot")
                        nc.vector.tensor_copy(out=o_t, in_=op)
                        r0 = (ch * 4 + tl) * 128
                        nc.sync.dma_start(out=out_d.ap()[r0:r0 + 128, :], in_=o_t)

    nc.compile()
    return nc


def _prep_shared(inputs):
    """Host-side layout prep shared across all 8 cores."""
    f32 = np.float32

    def bf(a):
        return np.ascontiguousarray(a.astype(BF16))

    cb = bf(inputs["char_codebook"])
    # keys: [k, d] -> [kt*128(d_inner) rows, (dt, k_inner) cols]
    mk = inputs["mem_keys"].reshape(KT, 128, DT, 128)
    keys = bf(mk.transpose(0, 3, 2, 1).reshape(K, D))
    vals = bf(inputs["mem_values"])
    # decay matrix (input-independent constant)
    i = np.arange(T, dtype=f32)
    diffs = np.clip(i[:, None] - i[None, :], 0.0, None).astype(f32)
    m = np.tril(np.float32(DECAY) ** diffs).astype(f32)
    dm = m / m.sum(axis=-1, keepdims=True)
    dmT = bf(dm.T)
    projw = bf(inputs["proj_w"])
    projb = bf(inputs["proj_b"].reshape(1, H))
    rese = bf(inputs["res_embed"])
    pos = np.ascontiguousarray(inputs["pos_embed"][:T].astype(f32))
    # fold layernorm gain/bias into w1/b1
    w1 = inputs["w1"].astype(f32)
    w1f = np.ascontiguousarray(inputs["ln_g"].astype(f32)[:, None] * w1)
    b1f = inputs["b1"].astype(f32) + inputs["ln_b"].astype(f32) @ w1

    def bias_tiles(b):
        return np.ascontiguousarray(b.reshape(HT, 128).T.astype(f32))

    return {
        "cb": cb, "keys": keys, "vals": vals, "dmT": dmT,
        "projw": projw, "projb": projb, "rese": rese, "pos": pos,
        "w1": w1f, "b1": bias_tiles(b1f),
        "w2": np.ascontiguousarray(inputs["w2"].astype(f32)),
        "b2": bias_tiles(inputs["b2"].astype(f32)),
        "w3": np.ascontiguousarray(inputs["w3"].astype(f32)),
        "b3": bias_tiles(inputs["b3"].astype(f32)),
        "hw": np.ascontiguousarray(inputs["head_w"].astype(f32)),
        "hb": np.ascontiguousarray(inputs["head_b"].reshape(1, V).astype(f32)),
    }


def make_in_maps(**inputs):
    shared = _prep_shared(inputs)
    idx = np.asarray(inputs["idx"], dtype=np.int32)
    in_maps = []
    for c in range(N_CORES):
        m = dict(shared)
        m["idx"] = np.ascontiguousarray(idx[c:c + 1, :])
        in_maps.append(m)
    return in_maps


def get_program():
    if "nc" not in _CACHE:
        _CACHE["nc"] = _build_program()
    return _CACHE["nc"]


def kernel(**inputs):
    from concourse import bass_utils
    nc = get_program()
    in_maps = make_in_maps(**inputs)
    res = bass_utils.run_bass_kernel_spmd(nc, in_maps,
                                          core_ids=list(range(N_CORES)))
    out = np.stack([res.results[c]["out"] for c in range(N_CORES)], axis=0)
    return out.astype(np.float32)
